# revision 13
# baseline (speedup 1.0000x reference)
"""Trainium2 Bass kernel for the attention-LSTM decoder step.

Reference computation (shapes: S=128 seq, N=256 batch, H=1024 hidden,
E=512 embed, V=32000 vocab, K=1024 energy channels):

  energy  = relu(cat(h0_rep, enc) @ W_energy.T + b_energy)   # only channel K-1
  attn    = softmax_s(energy[..., -1])                       # [S,N]
  ctx[n]  = sum_s attn[s,n] * enc[s,n,:]                     # [N,H]
  gates   = [ctx, emb[input]] @ W_ih.T + b_ih + h0 @ W_hh.T + b_hh
  c1      = sig(f)*c0 + sig(i)*tanh(g);  h1 = sig(o)*tanh(c1)
  preds   = h1 @ W_fc.T + b_fc                               # [N,V]

Key algebraic point: softmax only uses energy channel K-1, so the K x 2H
energy projection collapses to two dot products (w1 . h0[n], w2 . enc[s,n]).

Sharding over 8 cores:
  - attention: data-parallel over batch (each core: 32 batch rows, its
    16.8MB encoder shard), producing ctx^T h-major blocks
  - AllGather ctx -> LSTM tensor-parallel over hidden units (each core
    computes gate rows for its 128 hidden units for the full batch)
  - AllGather h1^T -> fc tensor-parallel over vocab (each core: 4000 rows
    of W_fc for the full batch)
Activations are kept feature-major ([feature, batch]) on device so the
contraction dim always lands on SBUF partitions.  The LSTM/fc matmuls run
in bf16 (f32 PSUM accumulation); attention stays f32.

Scheduling notes: attention is emitted in "waves" of 8 batch rows so the
DVE dot-product stream, the PE transpose/ctx-matmul stream and the enc DMA
stream stay concurrently busy instead of ping-ponging; a tiny dummy
AllGather is issued first so the one-time collectives init barrier runs
while only DMA/DVE have work; softmax skips the max-subtraction (energies
are O(1), exp cannot overflow).
"""

import numpy as np

import concourse.bass as bass
import concourse.bacc as bacc
import concourse.tile as tile
import concourse.mybir as mybir
from concourse.bass_utils import run_bass_kernel_spmd

S, N, H, E, V = 128, 256, 1024, 512, 32000
NC = 8
NL = N // NC          # 32 batch rows per core (attention phase)
HS = H // NC          # 128 hidden units per core (LSTM phase)
VS = V // NC          # 4000 vocab rows per core (fc phase)
CN = 4                # batch rows per enc DMA chunk
WV = 8                # batch rows per softmax/ctx wave (2 chunks)
NW = NL // WV         # 4 waves
VC = 500              # vocab cols per fc psum tile
NVC = VS // VC        # 8 vocab chunks
F32 = mybir.dt.float32
BF16 = mybir.dt.bfloat16
NPBF16 = mybir.dt.np(BF16)

_cache = {}


def _build():
    nc = bacc.Bacc("TRN2", target_bir_lowering=False, debug=False, num_devices=NC)

    # ---- per-core external inputs ----
    enc_d = nc.dram_tensor("enc", [S, NL, H], F32, kind="ExternalInput")
    w2rep_d = nc.dram_tensor("w2rep", [128, H], F32, kind="ExternalInput")
    w1c_d = nc.dram_tensor("w1c", [128, 8], F32, kind="ExternalInput")
    be_d = nc.dram_tensor("be", [1, 1], F32, kind="ExternalInput")
    h0lT_d = nc.dram_tensor("h0lT", [H, NL], F32, kind="ExternalInput")
    h0T_d = nc.dram_tensor("h0T", [H, N], BF16, kind="ExternalInput")
    embT_d = nc.dram_tensor("embT", [E, N], BF16, kind="ExternalInput")
    c0T_d = nc.dram_tensor("c0T", [HS, N], F32, kind="ExternalInput")
    wihT_d = nc.dram_tensor("wihT", [H + E, 4 * HS], BF16, kind="ExternalInput")
    whhT_d = nc.dram_tensor("whhT", [H, 4 * HS], BF16, kind="ExternalInput")
    bg_d = nc.dram_tensor("bg", [128, 4], F32, kind="ExternalInput")
    wfcT_d = nc.dram_tensor("wfcT", [H, VS], BF16, kind="ExternalInput")
    bfc_d = nc.dram_tensor("bfc", [1, VS], F32, kind="ExternalInput")

    # ---- per-core external outputs ----
    preds_d = nc.dram_tensor("preds", [N, VS], F32, kind="ExternalOutput")
    h1s_d = nc.dram_tensor("h1s", [HS, N], F32, kind="ExternalOutput")
    c1s_d = nc.dram_tensor("c1s", [HS, N], F32, kind="ExternalOutput")

    # ---- NEFF-embedded constants ----
    ident_d = nc.inline_tensor(np.eye(128, dtype=np.float32), name="ident")
    ones_d = nc.inline_tensor(np.ones((1, 128), dtype=np.float32), name="ones")
    onesbf_d = nc.inline_tensor(np.ones((1, 128), dtype=NPBF16), name="onesbf")

    with tile.TileContext(nc) as tc:
        with (
            tc.tile_pool(name="cpool", bufs=1) as cp,
            tc.tile_pool(name="encp", bufs=4) as encp,
            tc.tile_pool(name="scp", bufs=2) as scp,
            tc.tile_pool(name="wfcp", bufs=4) as wfcp,
            tc.tile_pool(name="obp", bufs=3) as obp,
            tc.tile_pool(name="smp", bufs=3) as smp,
            tc.tile_pool(name="dramp", bufs=1, space="DRAM") as dramp,
        ):
            # dummy collective first: absorbs the one-time comm-init barrier
            # while the engines have nothing but DMA/DVE work queued
            dum_in = dramp.tile([1, 4], F32)
            nc.gpsimd.dma_start(dum_in[:], w1c_d[0:1, 0:4])
            dum_out = dramp.tile([NC, 4], F32, addr_space="Shared")
            nc.gpsimd.collective_compute(
                "AllGather",
                mybir.AluOpType.bypass,
                replica_groups=[list(range(NC))],
                ins=[dum_in.opt()],
                outs=[dum_out.opt()],
            )

            # ---------- resident tiles ----------
            ident_sb = cp.tile([128, 128], F32)
            nc.sync.dma_start(ident_sb[:], ident_d[:])
            ones_sb = cp.tile([1, 128], F32)
            nc.sync.dma_start(ones_sb[:], ones_d[:])
            onesbf_sb = cp.tile([1, 128], BF16)
            nc.sync.dma_start(onesbf_sb[:], onesbf_d[:])
            w2rep_sb = cp.tile([128, H], F32)
            nc.sync.dma_start(w2rep_sb[:], w2rep_d[:])
            w1c_sb = cp.tile([128, 8], F32)
            nc.sync.dma_start(w1c_sb[:], w1c_d[:])
            be_sb = cp.tile([1, 1], F32)
            nc.sync.dma_start(be_sb[:], be_d[:])
            h0lT_sb = cp.tile([128, 8, NL], F32)
            nc.sync.dma_start(h0lT_sb[:], h0lT_d.rearrange("(k p) n -> p k n", p=128))

            h0T_sb = cp.tile([128, 8, N], BF16)
            nc.scalar.dma_start(h0T_sb[:], h0T_d.rearrange("(k p) n -> p k n", p=128))
            embT_sb = cp.tile([128, 4, N], BF16)
            nc.scalar.dma_start(embT_sb[:], embT_d.rearrange("(k p) n -> p k n", p=128))
            c0T_sb = cp.tile([128, N], F32)
            nc.scalar.dma_start(c0T_sb[:], c0T_d[:])
            wih_sb = cp.tile([128, 12, 4 * HS], BF16)
            nc.scalar.dma_start(wih_sb[:], wihT_d.rearrange("(k p) m -> p k m", p=128))
            whh_sb = cp.tile([128, 8, 4 * HS], BF16)
            nc.scalar.dma_start(whh_sb[:], whhT_d.rearrange("(k p) m -> p k m", p=128))
            bg_sb = cp.tile([128, 4], F32)
            nc.scalar.dma_start(bg_sb[:], bg_d[:])
            bfc_sb = cp.tile([1, VS], F32)
            nc.scalar.dma_start(bfc_sb[:], bfc_d[:])

            E_sb = cp.tile([128, NL], F32)       # energies [s, n_local]
            attn_sb = cp.tile([128, NL], F32)    # softmax weights [s, n_local]
            ctxT_sb = cp.tile([128, 8, NL], F32)

            with tc.tile_pool(name="ppa", bufs=1, space="PSUM") as ppa:
                # ---- e1[n] = w1 . h0_loc[n] + b_e, replicated to [128, NL] ----
                pe1 = ppa.tile([1, NL], F32, tag="e1")
                for kc in range(8):
                    nc.tensor.matmul(
                        pe1[:], w1c_sb[:, kc : kc + 1], h0lT_sb[:, kc, :],
                        start=(kc == 0), stop=False,
                    )
                nc.tensor.matmul(
                    pe1[:], be_sb[0:1, 0:1], ones_sb[0:1, 0:NL],
                    start=False, stop=True,
                )
                e1row_sb = cp.tile([1, NL], F32)
                nc.vector.tensor_copy(e1row_sb[:], pe1[:])
                pe1r = ppa.tile([128, NL], F32, tag="e1")
                nc.tensor.matmul(
                    pe1r[:], ones_sb[0:1, :], e1row_sb[0:1, :], start=True, stop=True
                )
                e1rep_sb = cp.tile([128, NL], F32)
                nc.vector.tensor_copy(e1rep_sb[:], pe1r[:])

                # ---- attention: 4 waves of 8 batch rows ----
                pctx = ppa.tile([128, 8, NL], F32, tag="ctx")
                ec_of = {}

                def softmax_ctx(w):
                    w0 = w * WV
                    pt = ppa.tile([WV, 128], F32, tag="t", bufs=2, name=f"pt{w}")
                    nc.tensor.transpose(pt[:], E_sb[:, w0 : w0 + WV], ident_sb[:])
                    et = smp.tile([WV, 128], F32, tag="et", name=f"et{w}")
                    nc.vector.tensor_scalar_max(et[:], pt[:], 0.0)
                    ex = smp.tile([WV, 128], F32, tag="ex", name=f"ex{w}")
                    den = smp.tile([WV, 1], F32, tag="den", name=f"den{w}")
                    nc.scalar.activation(
                        ex[:], et[:], mybir.ActivationFunctionType.Exp,
                        accum_out=den[:],
                    )
                    rden = smp.tile([WV, 1], F32, tag="rden", name=f"rden{w}")
                    nc.vector.reciprocal(rden[:], den[:])
                    at = smp.tile([WV, 128], F32, tag="at", name=f"at{w}")
                    nc.vector.tensor_scalar_mul(at[:], ex[:], rden[:])
                    pa = ppa.tile([128, WV], F32, tag="ta", bufs=2, name=f"pa{w}")
                    nc.tensor.transpose(pa[:], at[:], ident_sb[0:WV, 0:WV])
                    nc.scalar.copy(attn_sb[:, w0 : w0 + WV], pa[:])
                    for i in range(WV):
                        n = w0 + i
                        ecp, ii = ec_of[n]
                        for hc in range(8):
                            nc.tensor.matmul(
                                pctx[:, hc, n : n + 1],
                                ecp[:, ii, hc * 128 : (hc + 1) * 128],
                                attn_sb[:, n : n + 1],
                                start=True, stop=True,
                            )

                for w in range(NW):
                    for cc in range(2):
                        c = w * 2 + cc
                        n0 = c * CN
                        ec = encp.tile([128, CN, H], F32, tag="ec", name=f"ec{c}")
                        nc.sync.dma_start(ec[:], enc_d[:, n0 : n0 + CN, :])
                        for i in range(CN):
                            n = n0 + i
                            ec_of[n] = (ec, i)
                            sc = scp.tile([128, H], F32, tag="sc", name=f"sc{n}")
                            nc.vector.scalar_tensor_tensor(
                                sc[:], ec[:, i, :], 1.0, w2rep_sb[:],
                                op0=mybir.AluOpType.mult, op1=mybir.AluOpType.mult,
                                accum_out=E_sb[:, n : n + 1],
                            )
                    nc.vector.tensor_add(
                        E_sb[:, w * WV : (w + 1) * WV],
                        E_sb[:, w * WV : (w + 1) * WV],
                        e1rep_sb[:, w * WV : (w + 1) * WV],
                    )
                    if w >= 1:
                        softmax_ctx(w - 1)
                softmax_ctx(NW - 1)

                nc.vector.tensor_copy(ctxT_sb[:], pctx[:])

            # ---------- AllGather ctx^T ([H, NL] per core -> [NC*H, NL]) ----------
            ag_ctx_in = dramp.tile([H, NL], F32)
            nc.gpsimd.dma_start(
                ag_ctx_in.rearrange("(k p) n -> p k n", p=128), ctxT_sb[:]
            )
            ag_ctx_out = dramp.tile([NC * H, NL], F32, addr_space="Shared")
            nc.gpsimd.collective_compute(
                "AllGather",
                mybir.AluOpType.bypass,
                replica_groups=[list(range(NC))],
                ins=[ag_ctx_in.opt()],
                outs=[ag_ctx_out.opt()],
            )

            # reassemble ctx^T[h, n_global] as bf16 (block j = batch j*NL..)
            rnn_f32 = cp.tile([128, 8, N], F32)
            ctx_v = ag_ctx_out.rearrange("(j q) n -> q j n", j=NC)  # [H, NC, NL]
            for kc in range(8):
                nc.sync.dma_start(
                    rnn_f32[:, kc, :].rearrange("p (j n) -> p j n", j=NC),
                    ctx_v[kc * 128 : (kc + 1) * 128],
                )
            rnn_sb = cp.tile([128, 8, N], BF16)
            nc.vector.tensor_copy(rnn_sb[:], rnn_f32[:])

            # ---------- LSTM (tensor-parallel over 128 hidden units) ----------
            with tc.tile_pool(name="ppl", bufs=2, space="PSUM") as ppl:
                gact = []
                for g in range(4):
                    pg = ppl.tile([128, N], F32, tag="g", name=f"pg{g}")
                    m0 = g * 128
                    for kc in range(8):
                        nc.tensor.matmul(
                            pg[:], wih_sb[:, kc, m0 : m0 + 128], rnn_sb[:, kc, :],
                            start=(kc == 0), stop=False,
                        )
                    for kc in range(4):
                        nc.tensor.matmul(
                            pg[:], wih_sb[:, 8 + kc, m0 : m0 + 128], embT_sb[:, kc, :],
                            start=False, stop=False,
                        )
                    for kc in range(8):
                        nc.tensor.matmul(
                            pg[:], whh_sb[:, kc, m0 : m0 + 128], h0T_sb[:, kc, :],
                            start=False, stop=(kc == 7),
                        )
                    ga = cp.tile([128, N], F32, name=f"gact{g}")
                    func = (
                        mybir.ActivationFunctionType.Tanh
                        if g == 2
                        else mybir.ActivationFunctionType.Sigmoid
                    )
                    nc.scalar.activation(ga[:], pg[:], func, bias=bg_sb[:, g : g + 1])
                    gact.append(ga)

            si, sf, sg, so = gact
            t1 = cp.tile([128, N], F32)
            nc.vector.tensor_mul(t1[:], si[:], sg[:])
            t2 = cp.tile([128, N], F32)
            nc.vector.tensor_mul(t2[:], sf[:], c0T_sb[:])
            c1T = cp.tile([128, N], F32)
            nc.vector.tensor_add(c1T[:], t1[:], t2[:])
            tc1 = cp.tile([128, N], F32)
            nc.scalar.activation(tc1[:], c1T[:], mybir.ActivationFunctionType.Tanh)
            h1T = cp.tile([128, N], F32)
            nc.vector.tensor_mul(h1T[:], so[:], tc1[:])

            nc.sync.dma_start(c1s_d[:], c1T[:])
            nc.sync.dma_start(h1s_d[:], h1T[:])

            # ---------- AllGather h1^T ([HS, N] per core -> [H, N]) ----------
            ag_h1_in = dramp.tile([HS, N], F32)
            nc.gpsimd.dma_start(ag_h1_in[:], h1T[:])
            ag_h1_out = dramp.tile([H, N], F32, addr_space="Shared")
            nc.gpsimd.collective_compute(
                "AllGather",
                mybir.AluOpType.bypass,
                replica_groups=[list(range(NC))],
                ins=[ag_h1_in.opt()],
                outs=[ag_h1_out.opt()],
            )
            h1f_f32 = cp.tile([128, 8, N], F32)
            nc.sync.dma_start(
                h1f_f32[:], ag_h1_out.rearrange("(k p) n -> p k n", p=128)
            )
            h1f_sb = cp.tile([128, 8, N], BF16)
            nc.vector.tensor_copy(h1f_sb[:], h1f_f32[:])

            # ---------- fc (tensor-parallel over 4000 vocab rows) ----------
            with tc.tile_pool(name="ppf", bufs=3, space="PSUM") as ppf:
                for vc in range(NVC):
                    v0 = vc * VC
                    wt = wfcp.tile([128, 8, VC], BF16, tag="wfc", name=f"wt{vc}")
                    nc.scalar.dma_start(
                        wt[:],
                        wfcT_d[:, v0 : v0 + VC].rearrange("(k p) v -> p k v", p=128),
                    )
                    for bt in range(2):
                        pf = ppf.tile([128, VC], F32, tag="fc", name=f"pf{vc}_{bt}")
                        # seed psum with the bias via a K=1 matmul
                        nc.tensor.matmul(
                            pf[:], ones_sb[0:1, :], bfc_sb[0:1, v0 : v0 + VC],
                            start=True, stop=False,
                        )
                        for kc in range(8):
                            nc.tensor.matmul(
                                pf[:], h1f_sb[:, kc, bt * 128 : (bt + 1) * 128],
                                wt[:, kc, :], start=False, stop=(kc == 7),
                            )
                        ob = obp.tile([128, VC], F32, tag="ob", name=f"ob{vc}_{bt}")
                        nc.vector.tensor_copy(ob[:], pf[:])
                        nc.sync.dma_start(
                            preds_d[bt * 128 : (bt + 1) * 128, v0 : v0 + VC], ob[:]
                        )

    nc.compile()
    return nc


def _prep(input, encoder_states, hidden, cell, emb, W_energy, b_energy,
          W_ih, b_ih, W_hh, b_hh, W_fc, b_fc):
    f = np.float32
    input = np.asarray(input)
    enc = np.asarray(encoder_states, dtype=f)
    h0 = np.asarray(hidden, dtype=f)[0]          # [N,H]
    c0 = np.asarray(cell, dtype=f)[0]            # [N,H]
    emb = np.asarray(emb, dtype=f)
    W_energy = np.asarray(W_energy, dtype=f)
    b_energy = np.asarray(b_energy, dtype=f)
    W_ih = np.asarray(W_ih, dtype=f)
    b_ih = np.asarray(b_ih, dtype=f)
    W_hh = np.asarray(W_hh, dtype=f)
    b_hh = np.asarray(b_hh, dtype=f)
    W_fc = np.asarray(W_fc, dtype=f)
    b_fc = np.asarray(b_fc, dtype=f)

    emb_x = emb[input.astype(np.int64)]          # [N,E]
    embT = np.ascontiguousarray(emb_x.T).astype(NPBF16)
    w1 = W_energy[-1, :H]
    w2 = W_energy[-1, H:]
    w2rep = np.ascontiguousarray(np.broadcast_to(w2, (128, H)))
    w1c = np.ascontiguousarray(w1.reshape(8, 128).T)          # [128, 8]
    be = np.array([[b_energy[-1]]], dtype=f)
    h0T = np.ascontiguousarray(h0.T)
    h0T_bf = h0T.astype(NPBF16)
    bg_full = b_ih + b_hh

    in_maps = []
    for j in range(NC):
        rows = np.concatenate(
            [g * H + np.arange(j * HS, (j + 1) * HS) for g in range(4)]
        )
        in_maps.append({
            "enc": np.ascontiguousarray(enc[:, j * NL : (j + 1) * NL, :]),
            "w2rep": w2rep,
            "w1c": w1c,
            "be": be,
            "h0lT": np.ascontiguousarray(h0[j * NL : (j + 1) * NL].T),
            "h0T": h0T_bf,
            "embT": embT,
            "c0T": np.ascontiguousarray(c0[:, j * HS : (j + 1) * HS].T),
            "wihT": np.ascontiguousarray(W_ih[rows].T).astype(NPBF16),
            "whhT": np.ascontiguousarray(W_hh[rows].T).astype(NPBF16),
            "bg": np.ascontiguousarray(bg_full[rows].reshape(4, HS).T),
            "wfcT": np.ascontiguousarray(W_fc[j * VS : (j + 1) * VS].T).astype(NPBF16),
            "bfc": np.ascontiguousarray(b_fc[j * VS : (j + 1) * VS].reshape(1, VS)),
        })
    return in_maps


def _run(in_maps, trace=False):
    if "nc" not in _cache:
        _cache["nc"] = _build()
    nc = _cache["nc"]
    res = run_bass_kernel_spmd(nc, in_maps, core_ids=list(range(NC)), trace=trace)
    return res


def kernel(**inputs):
    in_maps = _prep(**inputs)
    res = _run(in_maps)
    results = res.results
    preds = np.concatenate([r["preds"] for r in results], axis=1)       # [N, V]
    h1T = np.concatenate([r["h1s"] for r in results], axis=0)           # [H, N]
    c1T = np.concatenate([r["c1s"] for r in results], axis=0)           # [H, N]
    h1 = np.ascontiguousarray(h1T.T)[None]
    c1 = np.ascontiguousarray(c1T.T)[None]
    return preds, h1, c1


# revision 15
# speedup vs baseline: 1.0725x; 1.0725x over previous
"""Trainium2 Bass kernel for the attention-LSTM decoder step.

Reference computation (shapes: S=128 seq, N=256 batch, H=1024 hidden,
E=512 embed, V=32000 vocab, K=1024 energy channels):

  energy  = relu(cat(h0_rep, enc) @ W_energy.T + b_energy)   # only channel K-1
  attn    = softmax_s(energy[..., -1])                       # [S,N]
  ctx[n]  = sum_s attn[s,n] * enc[s,n,:]                     # [N,H]
  gates   = [ctx, emb[input]] @ W_ih.T + b_ih + h0 @ W_hh.T + b_hh
  c1      = sig(f)*c0 + sig(i)*tanh(g);  h1 = sig(o)*tanh(c1)
  preds   = h1 @ W_fc.T + b_fc                               # [N,V]

Key algebraic point: softmax only uses energy channel K-1, so the K x 2H
energy projection collapses to two dot products (w1 . h0[n], w2 . enc[s,n]).

Sharding over 8 cores:
  - attention: data-parallel over batch (each core: 32 batch rows, its
    16.8MB encoder shard), producing ctx^T h-major blocks
  - AllGather ctx (bf16) -> LSTM tensor-parallel over hidden units (each
    core computes gate rows for its 128 hidden units for the full batch)
  - AllGather h1^T (bf16) -> fc tensor-parallel over vocab (each core:
    4000 rows of W_fc for the full batch)
Activations are kept feature-major ([feature, batch]) on device so the
contraction dim always lands on SBUF partitions.  The LSTM/fc matmuls run
in bf16 (f32 PSUM accumulation); attention math stays f32.

Scheduling notes:
  - attention runs in "waves" of 8 batch rows, softmax/ctx one wave behind
    the DVE dot-product stream, so DVE / PE / enc-DMA stay concurrently
    busy instead of ping-ponging (encp bufs covers 3 waves of liveness)
  - the e1 psum->sbuf copies go through the scalar engine so the DVE queue
    head is the enc dot products, not a wait on the tensor engine
  - a tiny dummy AllGather is issued first to absorb the one-time
    collectives init barrier / core-start skew
  - the LSTM gate psums accumulate their emb/h0 terms during attention
    (no AllGather dependency); only the 8 ctx terms wait on AllGather #1
  - both AllGathers move bf16 (their consumers are bf16 matmuls)
  - fc output copies alternate DVE/ACT and the preds writes alternate the
    two HWDGE rings so the 16-unit drain pipeline isn't one serial chain
  - softmax skips the max-subtraction (energies are O(1), exp safe)
"""

import numpy as np

import concourse.bass as bass
import concourse.bacc as bacc
import concourse.tile as tile
import concourse.mybir as mybir
from concourse.bass_utils import run_bass_kernel_spmd

S, N, H, E, V = 128, 256, 1024, 512, 32000
NC = 8
NL = N // NC          # 32 batch rows per core (attention phase)
HS = H // NC          # 128 hidden units per core (LSTM phase)
VS = V // NC          # 4000 vocab rows per core (fc phase)
CN = 4                # batch rows per enc DMA chunk
WV = 8                # batch rows per softmax/ctx wave (2 chunks)
NW = NL // WV         # 4 waves
VC = 500              # vocab cols per fc psum tile
NVC = VS // VC        # 8 vocab chunks
F32 = mybir.dt.float32
BF16 = mybir.dt.bfloat16
NPBF16 = mybir.dt.np(BF16)

_cache = {}


def _build():
    nc = bacc.Bacc("TRN2", target_bir_lowering=False, debug=False, num_devices=NC)

    # ---- per-core external inputs ----
    enc_d = nc.dram_tensor("enc", [S, NL, H], F32, kind="ExternalInput")
    w2rep_d = nc.dram_tensor("w2rep", [128, H], F32, kind="ExternalInput")
    w1c_d = nc.dram_tensor("w1c", [128, 8], F32, kind="ExternalInput")
    be_d = nc.dram_tensor("be", [1, 1], F32, kind="ExternalInput")
    h0lT_d = nc.dram_tensor("h0lT", [H, NL], F32, kind="ExternalInput")
    h0T_d = nc.dram_tensor("h0T", [H, N], BF16, kind="ExternalInput")
    embT_d = nc.dram_tensor("embT", [E, N], BF16, kind="ExternalInput")
    c0T_d = nc.dram_tensor("c0T", [HS, N], F32, kind="ExternalInput")
    wihT_d = nc.dram_tensor("wihT", [H + E, 4 * HS], BF16, kind="ExternalInput")
    whhT_d = nc.dram_tensor("whhT", [H, 4 * HS], BF16, kind="ExternalInput")
    bg_d = nc.dram_tensor("bg", [128, 4], F32, kind="ExternalInput")
    wfcT_d = nc.dram_tensor("wfcT", [H, VS], BF16, kind="ExternalInput")
    bfc_d = nc.dram_tensor("bfc", [1, VS], F32, kind="ExternalInput")

    # ---- per-core external outputs ----
    preds_d = nc.dram_tensor("preds", [N, VS], F32, kind="ExternalOutput")
    h1s_d = nc.dram_tensor("h1s", [HS, N], F32, kind="ExternalOutput")
    c1s_d = nc.dram_tensor("c1s", [HS, N], F32, kind="ExternalOutput")

    # ---- NEFF-embedded constants ----
    ident_d = nc.inline_tensor(np.eye(128, dtype=np.float32), name="ident")
    ones_d = nc.inline_tensor(np.ones((1, 128), dtype=np.float32), name="ones")
    onesbf_d = nc.inline_tensor(np.ones((1, 128), dtype=NPBF16), name="onesbf")

    with tile.TileContext(nc) as tc:
        with (
            tc.tile_pool(name="cpool", bufs=1) as cp,
            tc.tile_pool(name="encp", bufs=6) as encp,
            tc.tile_pool(name="scp", bufs=2) as scp,
            tc.tile_pool(name="wfcp", bufs=3) as wfcp,
            tc.tile_pool(name="obp", bufs=4) as obp,
            tc.tile_pool(name="smp", bufs=3) as smp,
            tc.tile_pool(name="dramp", bufs=1, space="DRAM") as dramp,
        ):
            # dummy collective first: absorbs the one-time comm-init barrier
            dum_in = dramp.tile([1, 4], F32)
            nc.gpsimd.dma_start(dum_in[:], w1c_d[0:1, 0:4])
            dum_out = dramp.tile([NC, 4], F32, addr_space="Shared")
            nc.gpsimd.collective_compute(
                "AllGather",
                mybir.AluOpType.bypass,
                replica_groups=[list(range(NC))],
                ins=[dum_in.opt()],
                outs=[dum_out.opt()],
            )

            # ---------- resident tiles ----------
            ident_sb = cp.tile([128, 128], F32)
            nc.sync.dma_start(ident_sb[:], ident_d[:])
            ones_sb = cp.tile([1, 128], F32)
            nc.sync.dma_start(ones_sb[:], ones_d[:])
            onesbf_sb = cp.tile([1, 128], BF16)
            nc.sync.dma_start(onesbf_sb[:], onesbf_d[:])
            w2rep_sb = cp.tile([128, H], F32)
            nc.sync.dma_start(w2rep_sb[:], w2rep_d[:])
            w1c_sb = cp.tile([128, 8], F32)
            nc.sync.dma_start(w1c_sb[:], w1c_d[:])
            be_sb = cp.tile([1, 1], F32)
            nc.sync.dma_start(be_sb[:], be_d[:])
            h0lT_sb = cp.tile([128, 8, NL], F32)
            nc.sync.dma_start(h0lT_sb[:], h0lT_d.rearrange("(k p) n -> p k n", p=128))

            h0T_sb = cp.tile([128, 8, N], BF16)
            nc.scalar.dma_start(h0T_sb[:], h0T_d.rearrange("(k p) n -> p k n", p=128))
            embT_sb = cp.tile([128, 4, N], BF16)
            nc.scalar.dma_start(embT_sb[:], embT_d.rearrange("(k p) n -> p k n", p=128))
            c0T_sb = cp.tile([128, N], F32)
            nc.scalar.dma_start(c0T_sb[:], c0T_d[:])
            wih_sb = cp.tile([128, 12, 4 * HS], BF16)
            nc.scalar.dma_start(wih_sb[:], wihT_d.rearrange("(k p) m -> p k m", p=128))
            whh_sb = cp.tile([128, 8, 4 * HS], BF16)
            nc.scalar.dma_start(whh_sb[:], whhT_d.rearrange("(k p) m -> p k m", p=128))
            bg_sb = cp.tile([128, 4], F32)
            nc.scalar.dma_start(bg_sb[:], bg_d[:])
            bfc_sb = cp.tile([1, VS], F32)
            nc.scalar.dma_start(bfc_sb[:], bfc_d[:])

            E_sb = cp.tile([128, NL], F32)       # energies [s, n_local]
            attn_sb = cp.tile([128, NL], F32)    # softmax weights [s, n_local]
            ctxT_bf = cp.tile([128, 8, NL], BF16)

            with (
                tc.tile_pool(name="ppa", bufs=1, space="PSUM") as ppa,
                tc.tile_pool(name="ppl", bufs=1, space="PSUM") as ppl,
            ):
                # ---- e1[n] = w1 . h0_loc[n] + b_e, replicated to [128, NL] ----
                pe1 = ppa.tile([1, NL], F32, tag="e1")
                for kc in range(8):
                    nc.tensor.matmul(
                        pe1[:], w1c_sb[:, kc : kc + 1], h0lT_sb[:, kc, :],
                        start=(kc == 0), stop=False,
                    )
                nc.tensor.matmul(
                    pe1[:], be_sb[0:1, 0:1], ones_sb[0:1, 0:NL],
                    start=False, stop=True,
                )
                e1row_sb = cp.tile([1, NL], F32)
                nc.scalar.copy(e1row_sb[:], pe1[:])
                pe1r = ppa.tile([128, NL], F32, tag="e1")
                nc.tensor.matmul(
                    pe1r[:], ones_sb[0:1, :], e1row_sb[0:1, :], start=True, stop=True
                )
                e1rep_sb = cp.tile([128, NL], F32)
                nc.scalar.copy(e1rep_sb[:], pe1r[:])

                # ---- attention: 4 waves of 8 batch rows ----
                pctx = ppa.tile([128, 8, NL], F32, tag="ctx")
                ec_of = {}

                def softmax_ctx(w):
                    w0 = w * WV
                    pt = ppa.tile([WV, 128], F32, tag="t", name=f"pt{w}")
                    nc.tensor.transpose(pt[:], E_sb[:, w0 : w0 + WV], ident_sb[:])
                    et = smp.tile([WV, 128], F32, tag="et", name=f"et{w}")
                    nc.vector.tensor_scalar_max(et[:], pt[:], 0.0)
                    ex = smp.tile([WV, 128], F32, tag="ex", name=f"ex{w}")
                    den = smp.tile([WV, 1], F32, tag="den", name=f"den{w}")
                    nc.scalar.activation(
                        ex[:], et[:], mybir.ActivationFunctionType.Exp,
                        accum_out=den[:],
                    )
                    rden = smp.tile([WV, 1], F32, tag="rden", name=f"rden{w}")
                    nc.vector.reciprocal(rden[:], den[:])
                    at = smp.tile([WV, 128], F32, tag="at", name=f"at{w}")
                    nc.vector.tensor_scalar_mul(at[:], ex[:], rden[:])
                    pa = ppa.tile([128, WV], F32, tag="ta", name=f"pa{w}")
                    nc.tensor.transpose(pa[:], at[:], ident_sb[0:WV, 0:WV])
                    nc.scalar.copy(attn_sb[:, w0 : w0 + WV], pa[:])
                    for i in range(WV):
                        n = w0 + i
                        ecp, ii = ec_of[n]
                        for hc in range(8):
                            nc.tensor.matmul(
                                pctx[:, hc, n : n + 1],
                                ecp[:, ii, hc * 128 : (hc + 1) * 128],
                                attn_sb[:, n : n + 1],
                                start=True, stop=True,
                            )

                for w in range(NW):
                    for cc in range(2):
                        c = w * 2 + cc
                        n0 = c * CN
                        ec = encp.tile([128, CN, H], F32, tag="ec", name=f"ec{c}")
                        nc.sync.dma_start(ec[:], enc_d[:, n0 : n0 + CN, :])
                        for i in range(CN):
                            n = n0 + i
                            ec_of[n] = (ec, i)
                            sc = scp.tile([128, H], BF16, tag="sc", name=f"sc{n}")
                            nc.vector.scalar_tensor_tensor(
                                sc[:], ec[:, i, :], 1.0, w2rep_sb[:],
                                op0=mybir.AluOpType.mult, op1=mybir.AluOpType.mult,
                                accum_out=E_sb[:, n : n + 1],
                            )
                    nc.vector.tensor_add(
                        E_sb[:, w * WV : (w + 1) * WV],
                        E_sb[:, w * WV : (w + 1) * WV],
                        e1rep_sb[:, w * WV : (w + 1) * WV],
                    )
                    if w >= 1:
                        softmax_ctx(w - 1)
                softmax_ctx(NW - 1)

                nc.vector.tensor_copy(ctxT_bf[:], pctx[:])

                # ---- LSTM gate psums: emb/h0 terms need no AllGather ----
                pgs = []
                for g in range(4):
                    pg = ppl.tile([128, N], F32, tag=f"g{g}", name=f"pg{g}")
                    m0 = g * 128
                    for kc in range(4):
                        nc.tensor.matmul(
                            pg[:], wih_sb[:, 8 + kc, m0 : m0 + 128], embT_sb[:, kc, :],
                            start=(kc == 0), stop=False,
                        )
                    for kc in range(8):
                        nc.tensor.matmul(
                            pg[:], whh_sb[:, kc, m0 : m0 + 128], h0T_sb[:, kc, :],
                            start=False, stop=False,
                        )
                    pgs.append(pg)

                # ---------- AllGather ctx^T in bf16 ----------
                ag_ctx_in = dramp.tile([H, NL], BF16)
                nc.gpsimd.dma_start(
                    ag_ctx_in.rearrange("(k p) n -> p k n", p=128), ctxT_bf[:]
                )
                ag_ctx_out = dramp.tile([NC * H, NL], BF16, addr_space="Shared")
                nc.gpsimd.collective_compute(
                    "AllGather",
                    mybir.AluOpType.bypass,
                    replica_groups=[list(range(NC))],
                    ins=[ag_ctx_in.opt()],
                    outs=[ag_ctx_out.opt()],
                )

                # reassemble ctx^T[h, n_global] (block j = batch j*NL..)
                rnn_sb = cp.tile([128, 8, N], BF16)
                ctx_v = ag_ctx_out.rearrange("(j q) n -> q j n", j=NC)
                for kc in range(8):
                    nc.sync.dma_start(
                        rnn_sb[:, kc, :].rearrange("p (j n) -> p j n", j=NC),
                        ctx_v[kc * 128 : (kc + 1) * 128],
                    )

                # ---- finish gates with the ctx terms ----
                gact = []
                for g in range(4):
                    pg = pgs[g]
                    m0 = g * 128
                    for kc in range(8):
                        nc.tensor.matmul(
                            pg[:], wih_sb[:, kc, m0 : m0 + 128], rnn_sb[:, kc, :],
                            start=False, stop=(kc == 7),
                        )
                    ga = cp.tile([128, N], F32, name=f"gact{g}")
                    func = (
                        mybir.ActivationFunctionType.Tanh
                        if g == 2
                        else mybir.ActivationFunctionType.Sigmoid
                    )
                    nc.scalar.activation(ga[:], pg[:], func, bias=bg_sb[:, g : g + 1])
                    gact.append(ga)

            si, sf, sg, so = gact
            t1 = cp.tile([128, N], F32)
            nc.vector.tensor_mul(t1[:], si[:], sg[:])
            t2 = cp.tile([128, N], F32)
            nc.vector.tensor_mul(t2[:], sf[:], c0T_sb[:])
            c1T = cp.tile([128, N], F32)
            nc.vector.tensor_add(c1T[:], t1[:], t2[:])
            tc1 = cp.tile([128, N], F32)
            nc.scalar.activation(tc1[:], c1T[:], mybir.ActivationFunctionType.Tanh)
            h1T = cp.tile([128, N], F32)
            nc.vector.tensor_mul(h1T[:], so[:], tc1[:])
            h1bf = cp.tile([128, N], BF16)
            nc.vector.tensor_copy(h1bf[:], h1T[:])

            nc.sync.dma_start(c1s_d[:], c1T[:])
            nc.sync.dma_start(h1s_d[:], h1T[:])

            # ---------- AllGather h1^T in bf16 ----------
            ag_h1_in = dramp.tile([HS, N], BF16)
            nc.gpsimd.dma_start(ag_h1_in[:], h1bf[:])
            ag_h1_out = dramp.tile([H, N], BF16, addr_space="Shared")
            nc.gpsimd.collective_compute(
                "AllGather",
                mybir.AluOpType.bypass,
                replica_groups=[list(range(NC))],
                ins=[ag_h1_in.opt()],
                outs=[ag_h1_out.opt()],
            )
            h1f_sb = cp.tile([128, 8, N], BF16)
            nc.sync.dma_start(
                h1f_sb[:], ag_h1_out.rearrange("(k p) n -> p k n", p=128)
            )

            # ---------- fc (tensor-parallel over 4000 vocab rows) ----------
            with tc.tile_pool(name="ppf", bufs=4, space="PSUM") as ppf:
                unit = 0
                for vc in range(NVC):
                    v0 = vc * VC
                    wt = wfcp.tile([128, 8, VC], BF16, tag="wfc", name=f"wt{vc}")
                    nc.scalar.dma_start(
                        wt[:],
                        wfcT_d[:, v0 : v0 + VC].rearrange("(k p) v -> p k v", p=128),
                    )
                    for bt in range(2):
                        pf = ppf.tile([128, VC], F32, tag="fc", name=f"pf{vc}_{bt}")
                        nc.tensor.matmul(
                            pf[:], ones_sb[0:1, :], bfc_sb[0:1, v0 : v0 + VC],
                            start=True, stop=False,
                        )
                        for kc in range(8):
                            nc.tensor.matmul(
                                pf[:], h1f_sb[:, kc, bt * 128 : (bt + 1) * 128],
                                wt[:, kc, :], start=False, stop=(kc == 7),
                            )
                        ob = obp.tile([128, VC], F32, tag="ob", name=f"ob{vc}_{bt}")
                        if unit % 2 == 0:
                            nc.vector.tensor_copy(ob[:], pf[:])
                            nc.sync.dma_start(
                                preds_d[bt * 128 : (bt + 1) * 128, v0 : v0 + VC],
                                ob[:],
                            )
                        else:
                            nc.scalar.copy(ob[:], pf[:])
                            nc.scalar.dma_start(
                                preds_d[bt * 128 : (bt + 1) * 128, v0 : v0 + VC],
                                ob[:],
                            )
                        unit += 1

    nc.compile()
    return nc


def _prep(input, encoder_states, hidden, cell, emb, W_energy, b_energy,
          W_ih, b_ih, W_hh, b_hh, W_fc, b_fc):
    f = np.float32
    input = np.asarray(input)
    enc = np.asarray(encoder_states, dtype=f)
    h0 = np.asarray(hidden, dtype=f)[0]          # [N,H]
    c0 = np.asarray(cell, dtype=f)[0]            # [N,H]
    emb = np.asarray(emb, dtype=f)
    W_energy = np.asarray(W_energy, dtype=f)
    b_energy = np.asarray(b_energy, dtype=f)
    W_ih = np.asarray(W_ih, dtype=f)
    b_ih = np.asarray(b_ih, dtype=f)
    W_hh = np.asarray(W_hh, dtype=f)
    b_hh = np.asarray(b_hh, dtype=f)
    W_fc = np.asarray(W_fc, dtype=f)
    b_fc = np.asarray(b_fc, dtype=f)

    emb_x = emb[input.astype(np.int64)]          # [N,E]
    embT = np.ascontiguousarray(emb_x.T).astype(NPBF16)
    w1 = W_energy[-1, :H]
    w2 = W_energy[-1, H:]
    w2rep = np.ascontiguousarray(np.broadcast_to(w2, (128, H)))
    w1c = np.ascontiguousarray(w1.reshape(8, 128).T)          # [128, 8]
    be = np.array([[b_energy[-1]]], dtype=f)
    h0T = np.ascontiguousarray(h0.T)
    h0T_bf = h0T.astype(NPBF16)
    bg_full = b_ih + b_hh

    in_maps = []
    for j in range(NC):
        rows = np.concatenate(
            [g * H + np.arange(j * HS, (j + 1) * HS) for g in range(4)]
        )
        in_maps.append({
            "enc": np.ascontiguousarray(enc[:, j * NL : (j + 1) * NL, :]),
            "w2rep": w2rep,
            "w1c": w1c,
            "be": be,
            "h0lT": np.ascontiguousarray(h0[j * NL : (j + 1) * NL].T),
            "h0T": h0T_bf,
            "embT": embT,
            "c0T": np.ascontiguousarray(c0[:, j * HS : (j + 1) * HS].T),
            "wihT": np.ascontiguousarray(W_ih[rows].T).astype(NPBF16),
            "whhT": np.ascontiguousarray(W_hh[rows].T).astype(NPBF16),
            "bg": np.ascontiguousarray(bg_full[rows].reshape(4, HS).T),
            "wfcT": np.ascontiguousarray(W_fc[j * VS : (j + 1) * VS].T).astype(NPBF16),
            "bfc": np.ascontiguousarray(b_fc[j * VS : (j + 1) * VS].reshape(1, VS)),
        })
    return in_maps


def _run(in_maps, trace=False):
    if "nc" not in _cache:
        _cache["nc"] = _build()
    nc = _cache["nc"]
    res = run_bass_kernel_spmd(nc, in_maps, core_ids=list(range(NC)), trace=trace)
    return res


def kernel(**inputs):
    in_maps = _prep(**inputs)
    res = _run(in_maps)
    results = res.results
    preds = np.concatenate([r["preds"] for r in results], axis=1)       # [N, V]
    h1T = np.concatenate([r["h1s"] for r in results], axis=0)           # [H, N]
    c1T = np.concatenate([r["c1s"] for r in results], axis=0)           # [H, N]
    h1 = np.ascontiguousarray(h1T.T)[None]
    c1 = np.ascontiguousarray(c1T.T)[None]
    return preds, h1, c1
